# revision 17
# baseline (speedup 1.0000x reference)
"""GATv2 (3-layer) distributed Trainium2 kernel — 8 NeuronCores.

Strategy (per sharding hint): nodes partitioned across 8 cores (balanced by
in-degree via snake assignment over degree-sorted nodes); edges partitioned by
destination so segment-softmax/aggregation stay core-local; per-layer
projections row-parallel with an AllGather of the (att-magnitude-scaled)
source-side projection per layer (layer 0 uses a host-replicated transposed
input instead, so no collective there).

Per-core edge phase: destination-stationary padded CSR. Destinations are
processed in groups of 128 (SBUF partitions); each group's incident edges are
processed in blocks of up to B source-slots per destination via indirect-DMA
row gathers. GATv2 math, with |att| folded into the projection weights on the
host (leaky_relu is positively homogeneous) and sign(att) applied as a +-1
multiply:
    alpha = sum_c sign(att)_c * prelu(xl'[src]_c + xr'[dst]_c, 0.2)
    p = exp(alpha + mask)            (no max-subtraction; range-safe here)
    out = (sum_e p_e * xl'[src_e]) / sum_e p_e / |att| + bias
"""
import sys

sys.path.insert(0, "/opt/trn_rl_repo")

import numpy as np

import concourse.bass as bass
import concourse.mybir as mybir
from concourse import bacc
from concourse.tile import TileContext
from concourse.tile_rust import add_dep_helper
from concourse.masks import make_identity
from concourse.bass_utils import run_bass_kernel_spmd

# ---------------- problem constants (hardcoded per spec) ----------------
N = 50000
E = 800000
IN = 256
HID = 64
HEADS = 4
HC = HEADS * HID  # 256
NCLS = 2
W = 8  # cores
P = 128
G = 49  # dst groups per core
NLOC = G * P  # 6272 padded local nodes per core
NFULL = W * NLOC  # 50176
B = 8  # edge-slot block for layers 0/1
B2 = 16  # edge-slot block for layer 2

F32 = mybir.dt.float32
BF16 = mybir.dt.bfloat16
I32 = mybir.dt.int32

NPBF16 = mybir.dt.np(BF16)


def _install_profile_hook():
    """Shim antenv.axon_hooks so trace=True can produce NTFF profiles."""
    import types

    if "antenv.axon_hooks" in sys.modules:
        return
    try:
        import trn_agent_boot.trn_boot as tb

        hook = tb._ntff_profile_via_ctypes("/opt/axon/libaxon_pjrt.so")
        m = types.ModuleType("antenv.axon_hooks")
        m.get_axon_ntff_profile_hook = lambda: hook
        sys.modules["antenv.axon_hooks"] = m
    except Exception:
        pass


# ---------------- host-side graph preprocessing ----------------

def _preprocess(edge_index: np.ndarray):
    src = np.concatenate([edge_index[0], np.arange(N, dtype=np.int64)])
    dst = np.concatenate([edge_index[1], np.arange(N, dtype=np.int64)])
    deg = np.bincount(dst, minlength=N)

    # snake assignment over degree-sorted nodes -> 6250 nodes/core, balanced degree
    order = np.argsort(-deg, kind="stable")
    ncore = N // W  # 6250
    core_of_rank = np.tile(
        np.concatenate([np.arange(W), np.arange(W)[::-1]]), (ncore + 1) // 2 + 1
    )[:N]
    node_core = np.empty(N, dtype=np.int64)
    node_core[order] = core_of_rank

    # local index: within each core, order nodes by degree descending
    gpos = np.empty(N, dtype=np.int64)  # node -> global padded position
    perm_nodes = np.full(NFULL, -1, dtype=np.int64)  # position -> node (real only)
    Dg_per_core = np.zeros((W, G), dtype=np.int64)
    for c in range(W):
        nodes_c = order[core_of_rank == c]  # degree-descending already
        li = np.arange(len(nodes_c))
        gpos[nodes_c] = c * NLOC + li
        perm_nodes[c * NLOC + li] = nodes_c
        dc = deg[nodes_c]
        for g in range(G):
            seg = dc[g * P : (g + 1) * P]
            Dg_per_core[c, g] = int(seg.max()) if len(seg) else 0
    Dg = Dg_per_core.max(axis=0)  # uniform across cores (SPMD program)
    Dg = np.maximum(Dg, 1)
    offs = np.concatenate([[0], np.cumsum(Dg)])  # group column offsets
    SLOTS = int(offs[-1])

    # slot index per edge (rank among edges sharing the same dst);
    # self-loops sort first so slot 0 is always the self edge (direct DMA)
    gd = gpos[dst]
    sort_i = np.lexsort((src != dst, gd))
    gd_s = gd[sort_i]
    starts = np.concatenate([[0], np.flatnonzero(np.diff(gd_s)) + 1])
    counts = np.diff(np.concatenate([starts, [len(gd_s)]]))
    slot_s = np.arange(len(gd_s)) - np.repeat(starts, counts)
    slot = np.empty(len(gd_s), dtype=np.int64)
    slot[sort_i] = slot_s

    # scatter edges into per-core [P, SLOTS] index/mask arrays
    ecore = gd // NLOC
    lidx = gd % NLOC
    erow = lidx % P
    egrp = lidx // P
    ecol = offs[egrp] + slot
    assert (slot < Dg[egrp]).all()

    idx_arr = np.zeros((W, P, SLOTS), dtype=np.int32)
    mask_arr = np.full((W, P, SLOTS), -100.0, dtype=np.float32)
    idx_arr[ecore, erow, ecol] = gpos[src].astype(np.int32)
    mask_arr[ecore, erow, ecol] = 0.0

    return {
        "gpos": gpos,
        "perm_nodes": perm_nodes,
        "Dg": Dg,
        "offs": offs,
        "SLOTS": SLOTS,
        "idx_arr": idx_arr,
        "mask_arr": mask_arr,
    }


# ---------------- device program ----------------

def _build_program(Dg, offs, SLOTS):
    nc = bacc.Bacc(None, target_bir_lowering=False)

    # ---- external inputs ----
    xT = nc.dram_tensor("xT", [IN, NFULL], BF16, kind="ExternalInput")
    xTloc = nc.dram_tensor("xTloc", [IN, NLOC], BF16, kind="ExternalInput")
    idx_in = nc.dram_tensor("idx", [P, SLOTS], I32, kind="ExternalInput")
    mask4_in = nc.dram_tensor("mask4", [P, SLOTS * HEADS], BF16, kind="ExternalInput")
    wl0_in = nc.dram_tensor("wl0", [IN, HC], BF16, kind="ExternalInput")
    wr0_in = nc.dram_tensor("wr0", [IN, HC], BF16, kind="ExternalInput")
    wl1_in = nc.dram_tensor("wl1", [HC, HC], BF16, kind="ExternalInput")
    wr1_in = nc.dram_tensor("wr1", [HC, HC], BF16, kind="ExternalInput")
    wl2_in = nc.dram_tensor("wl2", [HC, NCLS], BF16, kind="ExternalInput")
    wr2_in = nc.dram_tensor("wr2", [HC, NCLS], BF16, kind="ExternalInput")
    sig0_in = nc.dram_tensor("sig0", [P, HC], BF16, kind="ExternalInput")
    sig1_in = nc.dram_tensor("sig1", [P, HC], BF16, kind="ExternalInput")
    sig2_in = nc.dram_tensor("sig2", [P, NCLS], BF16, kind="ExternalInput")
    inv0_in = nc.dram_tensor("inv0", [P, HC], F32, kind="ExternalInput")
    inv1_in = nc.dram_tensor("inv1", [P, HC], F32, kind="ExternalInput")
    inv2_in = nc.dram_tensor("inv2", [P, NCLS], F32, kind="ExternalInput")
    b0_in = nc.dram_tensor("b0p", [P, HC], F32, kind="ExternalInput")  # b*|att|
    b1_in = nc.dram_tensor("b1p", [P, HC], F32, kind="ExternalInput")
    b2_in = nc.dram_tensor("b2p", [P, NCLS], F32, kind="ExternalInput")

    out_ext = nc.dram_tensor("out", [NLOC, NCLS], F32, kind="ExternalOutput")

    import os as _os
    _dbg = _os.environ.get("KERNEL_DEBUG", "0") in ("1", "2")
    if _dbg:
        dbg_xf0 = nc.dram_tensor("dbg_xf0", [NFULL, HC], BF16, kind="ExternalOutput")
        dbg_xr0 = nc.dram_tensor("dbg_xr0", [NLOC, HC], BF16, kind="ExternalOutput")
        dbg_h1T = nc.dram_tensor("dbg_h1T", [HC, NLOC], BF16, kind="ExternalOutput")
        dbg_xf1 = nc.dram_tensor("dbg_xf1", [NFULL, HC], BF16, kind="ExternalOutput")
        dbg_h2T = nc.dram_tensor("dbg_h2T", [HC, NLOC], BF16, kind="ExternalOutput")
        dbg_xf2 = nc.dram_tensor("dbg_xf2", [NFULL, NCLS], BF16, kind="ExternalOutput")

    # ---- internal DRAM ----
    xf0 = nc.dram_tensor("xf0", [NFULL, HC], BF16)  # xl0' full (replicated compute)
    xr0 = nc.dram_tensor("xr0", [NLOC, HC], BF16)
    xl0loc = nc.dram_tensor("xl0loc", [NLOC, HC], BF16)
    h1T = nc.dram_tensor("h1T", [HC, NLOC], BF16)
    xl1loc = nc.dram_tensor("xl1loc", [NLOC, HC], BF16)
    xf1 = nc.dram_tensor("xf1", [NFULL, HC], BF16, addr_space="Shared")
    xr1 = nc.dram_tensor("xr1", [NLOC, HC], BF16)
    h2T = nc.dram_tensor("h2T", [HC, NLOC], BF16)
    xl2loc = nc.dram_tensor("xl2loc", [NLOC, NCLS], BF16)
    xf2 = nc.dram_tensor("xf2", [NFULL, NCLS], BF16, addr_space="Shared")
    xr2 = nc.dram_tensor("xr2", [NLOC, NCLS], BF16)

    rg = [list(range(W))]

    with TileContext(nc) as tc:
        with (
            tc.tile_pool(name="const", bufs=1) as cpool,
            tc.tile_pool(name="proj", bufs=6) as ppool,
            tc.tile_pool(name="edge", bufs=4) as epool,
            tc.tile_pool(name="gath", bufs=6) as gpool,
            tc.tile_pool(name="acc", bufs=2) as apool,
            tc.tile_pool(name="psum", bufs=4, space="PSUM") as psum,
            tc.tile_pool(name="psumT", bufs=2, space="PSUM") as psumT,
        ):
            # ---- resident constants ----
            ident = cpool.tile([P, P], BF16)
            make_identity(nc, ident[:])

            def load_w(handle, kdim, ndim):
                tiles = []
                for k in range(0, kdim, P):
                    t = cpool.tile([P, ndim], BF16, tag=f"w_{handle.name}_{k}")
                    nc.sync.dma_start(out=t[:], in_=handle[k : k + P, :])
                    tiles.append(t)
                return tiles

            wl0_sb = load_w(wl0_in, IN, HC)
            wr0_sb = load_w(wr0_in, IN, HC)
            wl1_sb = load_w(wl1_in, HC, HC)
            wr1_sb = load_w(wr1_in, HC, HC)
            wl2_sb = load_w(wl2_in, HC, NCLS)
            wr2_sb = load_w(wr2_in, HC, NCLS)

            def bcast_row(handle, ndim, dt, tag):
                t = cpool.tile([P, ndim], dt, tag=tag)
                nc.sync.dma_start(out=t[:], in_=handle[:, :])
                return t

            sig0_t = bcast_row(sig0_in, HC, BF16, "sig0")
            sig1_t = bcast_row(sig1_in, HC, BF16, "sig1")
            sig2_t = bcast_row(sig2_in, NCLS, BF16, "sig2")
            inv0_t = bcast_row(inv0_in, HC, F32, "inv0")
            inv1_t = bcast_row(inv1_in, HC, F32, "inv1")
            inv2_t = bcast_row(inv2_in, NCLS, F32, "inv2")
            b0_t = bcast_row(b0_in, HC, F32, "b0")
            b1_t = bcast_row(b1_in, HC, F32, "b1")
            b2_t = bcast_row(b2_in, NCLS, F32, "b2")

            # ---- helper: projection of a span of node tiles ----
            def proj_span(srcT, t0, S, w_tiles_list, nout, out_drams):
                """lhsT chunks from srcT[:, t0:t0+S] (S a multiple of P); for
                each W, matmul-accumulate per 128-node subtile, write bf16
                tiles to out_drams. Returns write instrs per (dram, subtile)."""
                nk = len(w_tiles_list[0])
                lhs = []
                for k in range(nk):
                    a = ppool.tile([P, S], BF16, tag="lhs")
                    nc.sync.dma_start(
                        out=a[:], in_=srcT[k * P : (k + 1) * P, t0 : t0 + S]
                    )
                    lhs.append(a)
                winstrs = [[] for _ in out_drams]
                for j in range(S // P):
                    for wi, (w_tiles, odram) in enumerate(zip(w_tiles_list, out_drams)):
                        ps = psum.tile([P, nout], F32, tag="proj_ps")
                        for k, (a, wk) in enumerate(zip(lhs, w_tiles)):
                            nc.tensor.matmul(
                                ps[:], a[:, j * P : (j + 1) * P], wk[:],
                                start=(k == 0), stop=(k == nk - 1),
                            )
                        ob = ppool.tile([P, nout], BF16, tag="proj_out")
                        nc.scalar.copy(ob[:], ps[:])
                        winstrs[wi].append(
                            nc.sync.dma_start(
                                out=odram[t0 + j * P : t0 + (j + 1) * P, :], in_=ob[:]
                            )
                        )
                return winstrs

            # ---- phase A: layer-0 projections ----
            SPAN = 512
            for t0 in range(0, NFULL, SPAN):
                proj_span(xT, t0, min(SPAN, NFULL - t0), [wl0_sb], HC, [xf0])
            for t0 in range(0, NLOC, SPAN):
                proj_span(
                    xTloc, t0, min(SPAN, NLOC - t0), [wr0_sb, wl0_sb], HC, [xr0, xl0loc]
                )

            # ---- edge phase (shared for layers 0/1) ----
            DMAXG = int(Dg.max())

            def edge_phase(xf, xr, xloc, sig_t, inv_t, b_t, hT_out,
                           cc_dep=None, xr_deps=None, xloc_deps=None):
                """GATv2 message passing over dst groups; writes elu'd,
                transposed h tiles to hT_out."""
                for g in range(G):
                    D = int(Dg[g])
                    off = int(offs[g])
                    xr_t = epool.tile([P, HC], BF16, tag="xr")
                    _ld = nc.sync.dma_start(out=xr_t[:], in_=xr[g * P : (g + 1) * P, :])
                    if xr_deps is not None:
                        add_dep_helper(_ld.ins, xr_deps[g].ins, True, "xr RAW")
                    idxg = epool.tile([P, DMAXG], I32, tag="idxg")
                    nc.sync.dma_start(out=idxg[:, :D], in_=idx_in[:, off : off + D])
                    mkg = epool.tile([P, DMAXG * HEADS], BF16, tag="mkg")
                    nc.sync.dma_start(
                        out=mkg[:, : D * HEADS],
                        in_=mask4_in[:, off * HEADS : (off + D) * HEADS],
                    )
                    accs = apool.tile([P, B * HC], BF16, tag="accs")
                    nc.gpsimd.memset(accs[:], 0.0)
                    den = apool.tile([P, HEADS * B], F32, tag="den")
                    nc.gpsimd.memset(den[:], 0.0)

                    for d0 in range(0, D, B):
                        b = min(B, D - d0)
                        # prefill with xr, then gathers ACCUMULATE xl into it,
                        # so the tile holds q = xl[src] + xr[dst] directly.
                        xg = gpool.tile([P, B * HC], BF16, tag="xg")
                        nc.scalar.copy(
                            xg[:, : b * HC].rearrange("p (b c) -> p b c", b=b),
                            xr_t[:, None, :].to_broadcast([P, b, HC]),
                        )
                        for bb in range(b):
                            if d0 == 0 and bb == 0:
                                # slot 0 is the self-loop: contiguous local rows;
                                # q0 = xl_self + xr computed on DVE (overwrites
                                # the prefilled slot 0)
                                sl = epool.tile([P, HC], BF16, tag="sl")
                                _gi = nc.sync.dma_start(
                                    out=sl[:], in_=xloc[g * P : (g + 1) * P, :]
                                )
                                if xloc_deps is not None:
                                    add_dep_helper(
                                        _gi.ins, xloc_deps[g].ins, True, "selfloop RAW"
                                    )
                                nc.vector.tensor_tensor(
                                    out=xg[:, 0:HC], in0=sl[:], in1=xr_t[:],
                                    op=mybir.AluOpType.add,
                                )
                                continue
                            _gi = nc.gpsimd.indirect_dma_start(
                                out=xg[:, bb * HC : (bb + 1) * HC],
                                out_offset=None,
                                in_=xf[:, :],
                                in_offset=bass.IndirectOffsetOnAxis(
                                    ap=idxg[:, d0 + bb : d0 + bb + 1], axis=0
                                ),
                                compute_op=mybir.AluOpType.add,
                            )
                            if cc_dep is not None:
                                add_dep_helper(_gi.ins, cc_dep.ins, True, "gather RAW cc")
                        # v = prelu(q, 0.2) on ACT
                        v = epool.tile([P, B * HC], BF16, tag="v")
                        nc.scalar.activation(
                            v[:, : b * HC], xg[:, : b * HC],
                            mybir.ActivationFunctionType.Prelu, alpha=0.2,
                        )
                        # w = v * sign(att), in place
                        nc.vector.tensor_tensor(
                            out=v[:, : b * HC].rearrange("p (b c) -> p b c", b=b),
                            in0=v[:, : b * HC].rearrange("p (b c) -> p b c", b=b),
                            in1=sig_t[:, None, :].to_broadcast([P, b, HC]),
                            op=mybir.AluOpType.mult,
                        )
                        # alpha[p, b, h] = sum_c w
                        al = epool.tile([P, B * HEADS], F32, tag="al")
                        nc.vector.tensor_reduce(
                            out=al[:, : b * HEADS].rearrange("p (b h) -> p b h", h=HEADS),
                            in_=v[:, : b * HC].rearrange(
                                "p (b h c) -> p b h c", h=HEADS, c=HID
                            ),
                            axis=mybir.AxisListType.X,
                            op=mybir.AluOpType.add,
                        )
                        # alpha += mask (from the group-resident mask tile)
                        nc.vector.tensor_tensor(
                            out=al[:, : b * HEADS], in0=al[:, : b * HEADS],
                            in1=mkg[:, d0 * HEADS : (d0 + b) * HEADS],
                            op=mybir.AluOpType.add,
                        )
                        # p = exp(alpha), replicated across channels (ACT
                        # broadcast-in writes each exp 64x -> matches q layout)
                        pr = epool.tile([P, B * HC], BF16, tag="pr")
                        nc.scalar.activation(
                            pr[:, : b * HC].rearrange("p (bh c) -> p bh c", c=HID),
                            al[:, : b * HEADS][:, :, None].to_broadcast(
                                [P, b * HEADS, HID]
                            ),
                            mybir.ActivationFunctionType.Exp,
                        )
                        # den[h, b] += p[b, h] from the replicated tile (stride HID)
                        nc.vector.tensor_tensor(
                            out=den[:].rearrange("p (h b) -> p h b", h=HEADS)[:, :, :b],
                            in0=den[:].rearrange("p (h b) -> p h b", h=HEADS)[:, :, :b],
                            in1=pr[:, : b * HC]
                            .rearrange("p (b h c) -> p b h c", h=HEADS, c=HID)[
                                :, :, :, 0:1
                            ]
                            .rearrange("p b h x -> p h (b x)"),
                            op=mybir.AluOpType.add,
                        )
                        # t = q * p  (both bf16 unit-stride -> 2x mode)
                        tt = epool.tile([P, B * HC], BF16, tag="tt")
                        nc.vector.tensor_tensor(
                            out=tt[:, : b * HC], in0=xg[:, : b * HC],
                            in1=pr[:, : b * HC], op=mybir.AluOpType.mult,
                        )
                        # slot-wise accumulate (bf16 2x mode)
                        nc.vector.tensor_tensor(
                            out=accs[:, : b * HC], in0=accs[:, : b * HC],
                            in1=tt[:, : b * HC], op=mybir.AluOpType.add,
                        )

                    # ---- group end ----
                    acc = apool.tile([P, HC], F32, tag="acc")
                    nc.vector.tensor_reduce(
                        out=acc[:].rearrange("p (c x) -> p c x", x=1),
                        in_=accs[:].rearrange("p (b c) -> p c b", b=B),
                        axis=mybir.AxisListType.X,
                        op=mybir.AluOpType.add,
                    )
                    dsum = apool.tile([P, HEADS], F32, tag="dsum")
                    nc.vector.tensor_reduce(
                        out=dsum[:].rearrange("p (h x) -> p h x", x=1),
                        in_=den[:].rearrange("p (h b) -> p h b", h=HEADS),
                        axis=mybir.AxisListType.X,
                        op=mybir.AluOpType.add,
                    )
                    nc.vector.tensor_scalar_add(dsum[:], dsum[:], 1e-16)
                    rec = apool.tile([P, HEADS], F32, tag="rec")
                    nc.vector.reciprocal(rec[:], dsum[:])
                    # acc = sum p*(xl+xr); subtract (sum p) * xr to recover sum p*xl
                    cor = apool.tile([P, HC], F32, tag="cor")
                    nc.vector.tensor_tensor(
                        out=cor[:].rearrange("p (h c) -> p h c", h=HEADS),
                        in0=xr_t[:].rearrange("p (h c) -> p h c", h=HEADS),
                        in1=dsum[:, :, None].to_broadcast([P, HEADS, HID]),
                        op=mybir.AluOpType.mult,
                    )
                    nc.vector.tensor_tensor(
                        out=acc[:], in0=acc[:], in1=cor[:],
                        op=mybir.AluOpType.subtract,
                    )
                    on = apool.tile([P, HC], F32, tag="on")
                    nc.vector.tensor_tensor(
                        out=on[:].rearrange("p (h c) -> p h c", h=HEADS),
                        in0=acc[:].rearrange("p (h c) -> p h c", h=HEADS),
                        in1=rec[:, :, None].to_broadcast([P, HEADS, HID]),
                        op=mybir.AluOpType.mult,
                    )
                    # + b*|att|, then / |att|
                    nc.vector.tensor_tensor(
                        out=on[:], in0=on[:], in1=b_t[:], op=mybir.AluOpType.add
                    )
                    nc.vector.tensor_tensor(
                        out=on[:], in0=on[:], in1=inv_t[:], op=mybir.AluOpType.mult
                    )
                    # elu -> bf16 h tile
                    mn = apool.tile([P, HC], F32, tag="mn")
                    nc.vector.tensor_scalar_min(mn[:], on[:], 0.0)
                    em = apool.tile([P, HC], F32, tag="em")
                    nc.scalar.activation(em[:], mn[:], mybir.ActivationFunctionType.Exp)
                    rl = apool.tile([P, HC], F32, tag="rl")
                    nc.vector.tensor_scalar_max(rl[:], on[:], 0.0)
                    hb = apool.tile([P, HC], BF16, tag="hb")
                    nc.vector.scalar_tensor_tensor(
                        out=hb[:], in0=em[:], scalar=-1.0, in1=rl[:],
                        op0=mybir.AluOpType.add, op1=mybir.AluOpType.add,
                    )
                    # transpose h tile -> hT_out[:, g*P:(g+1)*P]
                    for k in range(HC // P):
                        tp = psumT.tile([P, P], BF16, tag="tp")
                        nc.tensor.transpose(
                            out=tp[:], in_=hb[:, k * P : (k + 1) * P], identity=ident[:]
                        )
                        tb = ppool.tile([P, P], BF16, tag="tb")
                        nc.scalar.copy(tb[:], tp[:])
                        nc.sync.dma_start(
                            out=hT_out[k * P : (k + 1) * P, g * P : (g + 1) * P],
                            in_=tb[:],
                        )

            # ---- barrier: xf0/xr0 writes -> edge-phase gathers ----
            tc.strict_bb_all_engine_barrier()

            # ---- layer 0 ----
            edge_phase(xf0, xr0, xl0loc, sig0_t, inv0_t, b0_t, h1T)

            # ---- barrier: h1T writes -> layer-1 projection reads ----
            tc.strict_bb_all_engine_barrier()

            # ---- layer 1 projections + AllGather ----
            xr1_w = []
            xl1_w = []
            for t0 in range(0, NLOC, SPAN):
                ws = proj_span(
                    h1T, t0, min(SPAN, NLOC - t0), [wl1_sb, wr1_sb], HC, [xl1loc, xr1]
                )
                xl1_w.extend(ws[0])
                xr1_w.extend(ws[1])
            cc1 = nc.gpsimd.collective_compute(
                "AllGather", mybir.AluOpType.bypass, replica_groups=rg,
                ins=[xl1loc[:, :]], outs=[xf1[:, :]],
            )

            # ---- layer 1 ----
            edge_phase(xf1, xr1, xl1loc, sig1_t, inv1_t, b1_t, h2T,
                       cc_dep=cc1, xr_deps=xr1_w, xloc_deps=xl1_w)

            # ---- barrier: h2T writes -> layer-2 projection reads ----
            tc.strict_bb_all_engine_barrier()

            # ---- layer 2 projections + AllGather ----
            xr2_w = []
            xl2_w = []
            for t0 in range(0, NLOC, SPAN):
                ws = proj_span(
                    h2T, t0, min(SPAN, NLOC - t0), [wl2_sb, wr2_sb], NCLS, [xl2loc, xr2]
                )
                xl2_w.extend(ws[0])
                xr2_w.extend(ws[1])
            cc2 = nc.gpsimd.collective_compute(
                "AllGather", mybir.AluOpType.bypass, replica_groups=rg,
                ins=[xl2loc[:, :]], outs=[xf2[:, :]],
            )

            # ---- layer 2 edge phase (H=1, C=2) ----
            for g in range(G):
                D = int(Dg[g])
                off = int(offs[g])
                xr_t = epool.tile([P, NCLS], BF16, tag="xr2")
                _ld2 = nc.sync.dma_start(out=xr_t[:], in_=xr2[g * P : (g + 1) * P, :])
                add_dep_helper(_ld2.ins, xr2_w[g].ins, True, "xr2 RAW")
                idxg2 = epool.tile([P, DMAXG], I32, tag="idxg2")
                nc.sync.dma_start(out=idxg2[:, :D], in_=idx_in[:, off : off + D])
                mkg2 = epool.tile([P, DMAXG], BF16, tag="mkg2")
                nc.sync.dma_start(
                    out=mkg2[:, :D],
                    in_=mask4_in[:]
                    .rearrange("p (s h) -> p s h", h=HEADS)[:, off : off + D, 0:1]
                    .rearrange("p s x -> p (s x)"),
                )
                acc2 = apool.tile([P, NCLS * B2], F32, tag="acc2")
                nc.gpsimd.memset(acc2[:], 0.0)
                den2 = apool.tile([P, B2], F32, tag="den2")
                nc.gpsimd.memset(den2[:], 0.0)

                for d0 in range(0, D, B2):
                    b = min(B2, D - d0)
                    xg = gpool.tile([P, B2 * NCLS], BF16, tag="xg2")
                    nc.scalar.copy(
                        xg[:, : b * NCLS].rearrange("p (b c) -> p b c", b=b),
                        xr_t[:, None, :].to_broadcast([P, b, NCLS]),
                    )
                    for bb in range(b):
                        if d0 == 0 and bb == 0:
                            sl2 = epool.tile([P, NCLS], BF16, tag="sl2")
                            _gi2 = nc.sync.dma_start(
                                out=sl2[:], in_=xl2loc[g * P : (g + 1) * P, :]
                            )
                            add_dep_helper(_gi2.ins, xl2_w[g].ins, True, "selfloop2 RAW")
                            nc.vector.tensor_tensor(
                                out=xg[:, 0:NCLS], in0=sl2[:], in1=xr_t[:],
                                op=mybir.AluOpType.add,
                            )
                            continue
                        _gi2 = nc.gpsimd.indirect_dma_start(
                            out=xg[:, bb * NCLS : (bb + 1) * NCLS],
                            out_offset=None,
                            in_=xf2[:, :],
                            in_offset=bass.IndirectOffsetOnAxis(
                                ap=idxg2[:, d0 + bb : d0 + bb + 1], axis=0
                            ),
                            compute_op=mybir.AluOpType.add,
                        )
                        add_dep_helper(_gi2.ins, cc2.ins, True, "gather RAW cc2")
                    v = epool.tile([P, B2 * NCLS], BF16, tag="v2")
                    nc.scalar.activation(
                        v[:, : b * NCLS], xg[:, : b * NCLS],
                        mybir.ActivationFunctionType.Prelu, alpha=0.2,
                    )
                    nc.vector.tensor_tensor(
                        out=v[:, : b * NCLS].rearrange("p (b c) -> p b c", b=b),
                        in0=v[:, : b * NCLS].rearrange("p (b c) -> p b c", b=b),
                        in1=sig2_t[:, None, :].to_broadcast([P, b, NCLS]),
                        op=mybir.AluOpType.mult,
                    )
                    al = epool.tile([P, B2], F32, tag="al2")
                    nc.vector.tensor_reduce(
                        out=al[:, :b].rearrange("p (b x) -> p b x", x=1),
                        in_=v[:, : b * NCLS].rearrange("p (b c) -> p b c", b=b),
                        axis=mybir.AxisListType.X,
                        op=mybir.AluOpType.add,
                    )
                    nc.vector.tensor_tensor(
                        out=al[:, :b], in0=al[:, :b],
                        in1=mkg2[:, d0 : d0 + b],
                        op=mybir.AluOpType.add,
                    )
                    pe = epool.tile([P, B2], F32, tag="pe2")
                    nc.scalar.activation(
                        pe[:, :b], al[:, :b], mybir.ActivationFunctionType.Exp
                    )
                    nc.vector.tensor_tensor(
                        out=den2[:, :b], in0=den2[:, :b], in1=pe[:, :b],
                        op=mybir.AluOpType.add,
                    )
                    # acc2[c, b] += xg[b, c] * p[b]
                    tt = epool.tile([P, B2 * NCLS], F32, tag="tt2")
                    nc.vector.tensor_tensor(
                        out=tt[:, : b * NCLS].rearrange("p (b c) -> p b c", b=b),
                        in0=xg[:, : b * NCLS].rearrange("p (b c) -> p b c", b=b),
                        in1=pe[:, :b, None].to_broadcast([P, b, NCLS]),
                        op=mybir.AluOpType.mult,
                    )
                    nc.vector.tensor_tensor(
                        out=acc2[:].rearrange("p (c b) -> p c b", c=NCLS)[:, :, :b],
                        in0=acc2[:].rearrange("p (c b) -> p c b", c=NCLS)[:, :, :b],
                        in1=tt[:, : b * NCLS].rearrange("p (b c) -> p c b", b=b),
                        op=mybir.AluOpType.add,
                    )

                dsum = apool.tile([P, 1], F32, tag="dsum2")
                nc.vector.tensor_reduce(
                    out=dsum[:], in_=den2[:], axis=mybir.AxisListType.X,
                    op=mybir.AluOpType.add,
                )
                nc.vector.tensor_scalar_add(dsum[:], dsum[:], 1e-16)
                rec = apool.tile([P, 1], F32, tag="rec2")
                nc.vector.reciprocal(rec[:], dsum[:])
                o2 = apool.tile([P, NCLS], F32, tag="o2")
                nc.vector.tensor_reduce(
                    out=o2[:].rearrange("p (c x) -> p c x", x=1),
                    in_=acc2[:].rearrange("p (c b) -> p c b", c=NCLS),
                    axis=mybir.AxisListType.X,
                    op=mybir.AluOpType.add,
                )
                # o2 = sum p*(xl+xr); subtract (sum p)*xr
                cor2 = apool.tile([P, NCLS], F32, tag="cor2")
                nc.vector.tensor_scalar(
                    out=cor2[:], in0=xr_t[:], scalar1=dsum[:, :1], scalar2=None,
                    op0=mybir.AluOpType.mult,
                )
                nc.vector.tensor_tensor(
                    out=o2[:], in0=o2[:], in1=cor2[:], op=mybir.AluOpType.subtract
                )
                nc.vector.tensor_scalar(
                    out=o2[:], in0=o2[:], scalar1=rec[:, :1], scalar2=None,
                    op0=mybir.AluOpType.mult,
                )
                nc.vector.tensor_tensor(
                    out=o2[:], in0=o2[:], in1=b2_t[:], op=mybir.AluOpType.add
                )
                nc.vector.tensor_tensor(
                    out=o2[:], in0=o2[:], in1=inv2_t[:], op=mybir.AluOpType.mult
                )
                nc.sync.dma_start(out=out_ext[g * P : (g + 1) * P, :], in_=o2[:])

            if _dbg:
                nc.sync.dma_start(out=dbg_xf0[:, :], in_=xf0[:, :])
                nc.sync.dma_start(out=dbg_xr0[:, :], in_=xr0[:, :])
                nc.sync.dma_start(out=dbg_h1T[:, :], in_=h1T[:, :])
                _dx1 = nc.sync.dma_start(out=dbg_xf1[:, :], in_=xf1[:, :])
                add_dep_helper(_dx1.ins, cc1.ins, True, "dbg xf1")
                nc.sync.dma_start(out=dbg_h2T[:, :], in_=h2T[:, :])
                _dx2 = nc.sync.dma_start(out=dbg_xf2[:, :], in_=xf2[:, :])
                add_dep_helper(_dx2.ins, cc2.ins, True, "dbg xf2")

    nc.compile()
    return nc


# ---------------- top-level entry ----------------

def kernel(**inputs) -> np.ndarray:
    _install_profile_hook()

    x = np.asarray(inputs["x"], dtype=np.float32)
    edge_index = np.asarray(inputs["edge_index"])
    pre = _preprocess(np.asarray(edge_index, dtype=np.int64))

    gpos, perm_nodes = pre["gpos"], pre["perm_nodes"]
    Dg, offs, SLOTS = pre["Dg"], pre["offs"], pre["SLOTS"]
    idx_arr, mask_arr = pre["idx_arr"], pre["mask_arr"]

    # host-side weight folding
    def fold(att):
        a = np.asarray(att, dtype=np.float32).reshape(-1)
        mag = np.abs(a)
        sig = np.sign(a)
        bad = mag < 1e-12
        mag[bad] = 1.0
        sig[bad] = 0.0
        return mag, sig

    mag0, sg0 = fold(inputs["att0"])
    mag1, sg1 = fold(inputs["att1"])
    mag2, sg2 = fold(inputs["att2"])

    wl0 = (np.asarray(inputs["Wl0"], np.float32) * mag0[None, :]).astype(NPBF16)
    wr0 = (np.asarray(inputs["Wr0"], np.float32) * mag0[None, :]).astype(NPBF16)
    wl1 = (np.asarray(inputs["Wl1"], np.float32) * mag1[None, :]).astype(NPBF16)
    wr1 = (np.asarray(inputs["Wr1"], np.float32) * mag1[None, :]).astype(NPBF16)
    wl2 = (np.asarray(inputs["Wl2"], np.float32) * mag2[None, :]).astype(NPBF16)
    wr2 = (np.asarray(inputs["Wr2"], np.float32) * mag2[None, :]).astype(NPBF16)

    b0p = (np.asarray(inputs["b0"], np.float32) * mag0)[None, :].astype(np.float32)
    b1p = (np.asarray(inputs["b1"], np.float32) * mag1)[None, :].astype(np.float32)
    b2p = (np.asarray(inputs["b2"], np.float32) * mag2)[None, :].astype(np.float32)

    # permuted, padded, transposed input
    x_perm = np.zeros((NFULL, IN), dtype=np.float32)
    real = perm_nodes >= 0
    x_perm[real] = x[perm_nodes[real]]
    xT_np = np.ascontiguousarray(x_perm.T).astype(NPBF16)

    common = {
        "xT": xT_np,
        "wl0": wl0, "wr0": wr0, "wl1": wl1, "wr1": wr1, "wl2": wl2, "wr2": wr2,
        "sig0": np.broadcast_to(sg0[None, :], (P, HC)).astype(NPBF16),
        "sig1": np.broadcast_to(sg1[None, :], (P, HC)).astype(NPBF16),
        "sig2": np.broadcast_to(sg2[None, :], (P, NCLS)).astype(NPBF16),
        "inv0": np.ascontiguousarray(np.broadcast_to((1.0 / mag0)[None, :], (P, HC))).astype(np.float32),
        "inv1": np.ascontiguousarray(np.broadcast_to((1.0 / mag1)[None, :], (P, HC))).astype(np.float32),
        "inv2": np.ascontiguousarray(np.broadcast_to((1.0 / mag2)[None, :], (P, NCLS))).astype(np.float32),
        "b0p": np.ascontiguousarray(np.broadcast_to(b0p, (P, HC))).astype(np.float32),
        "b1p": np.ascontiguousarray(np.broadcast_to(b1p, (P, HC))).astype(np.float32),
        "b2p": np.ascontiguousarray(np.broadcast_to(b2p, (P, NCLS))).astype(np.float32),
    }
    in_maps = []
    for c in range(W):
        m = dict(common)
        m["xTloc"] = np.ascontiguousarray(xT_np[:, c * NLOC : (c + 1) * NLOC])
        m["idx"] = idx_arr[c]
        m["mask4"] = np.repeat(mask_arr[c], HEADS, axis=1).astype(NPBF16)
        in_maps.append(m)

    nc = _build_program(Dg, offs, SLOTS)
    import os
    trace = os.environ.get("KERNEL_TRACE", "0") == "1"
    res = run_bass_kernel_spmd(nc, in_maps, list(range(W)), trace=trace)
    kernel.last_exec_time_ns = res.exec_time_ns
    kernel.last_results = res.results
    kernel.last_pre = pre
    kernel.last_in_maps = in_maps

    out_full = np.zeros((N, NCLS), dtype=np.float32)
    for c in range(W):
        o = res.results[c]["out"]  # [NLOC, 2]
        sel = perm_nodes[c * NLOC : (c + 1) * NLOC]
        r = sel >= 0
        out_full[sel[r]] = o[r]
    return out_full


if __name__ == "__main__":
    rng = np.random.default_rng(0)
    x = rng.standard_normal((N, IN)).astype(np.float32)
    ei = rng.integers(0, N, (2, E)).astype(np.int64)
    print("smoke build only")


# revision 18
# speedup vs baseline: 1.3114x; 1.3114x over previous
"""GATv2 (3-layer) distributed Trainium2 kernel — 8 NeuronCores.

Strategy (per sharding hint): nodes partitioned across 8 cores (balanced by
in-degree via snake assignment over degree-sorted nodes); edges partitioned by
destination so segment-softmax/aggregation stay core-local; per-layer
projections row-parallel with an AllGather of the (att-magnitude-scaled)
source-side projection per layer (layer 0 uses a host-replicated transposed
input instead, so no collective there).

Per-core edge phase: destination-stationary padded CSR. Destinations are
processed in groups of 128 (SBUF partitions); each group's incident edges are
processed in blocks of up to B source-slots per destination via indirect-DMA
row gathers. GATv2 math, with |att| folded into the projection weights on the
host (leaky_relu is positively homogeneous) and sign(att) applied as a +-1
multiply:
    alpha = sum_c sign(att)_c * prelu(xl'[src]_c + xr'[dst]_c, 0.2)
    p = exp(alpha + mask)            (no max-subtraction; range-safe here)
    out = (sum_e p_e * xl'[src_e]) / sum_e p_e / |att| + bias
"""
import sys

sys.path.insert(0, "/opt/trn_rl_repo")

import numpy as np

import concourse.bass as bass
import concourse.mybir as mybir
from concourse import bacc
from concourse.tile import TileContext
from concourse.tile_rust import add_dep_helper
from concourse.masks import make_identity
from concourse.bass_utils import run_bass_kernel_spmd

# ---------------- problem constants (hardcoded per spec) ----------------
N = 50000
E = 800000
IN = 256
HID = 64
HEADS = 4
HC = HEADS * HID  # 256
NCLS = 2
W = 8  # cores
P = 128
G = 49  # dst groups per core
NLOC = G * P  # 6272 padded local nodes per core
NFULL = W * NLOC  # 50176
B = 8  # edge-slot block for layers 0/1
B2 = 16  # edge-slot block for layer 2

F32 = mybir.dt.float32
BF16 = mybir.dt.bfloat16
I32 = mybir.dt.int32

NPBF16 = mybir.dt.np(BF16)


def _install_profile_hook():
    """Shim antenv.axon_hooks so trace=True can produce NTFF profiles."""
    import types

    if "antenv.axon_hooks" in sys.modules:
        return
    try:
        import trn_agent_boot.trn_boot as tb

        hook = tb._ntff_profile_via_ctypes("/opt/axon/libaxon_pjrt.so")
        m = types.ModuleType("antenv.axon_hooks")
        m.get_axon_ntff_profile_hook = lambda: hook
        sys.modules["antenv.axon_hooks"] = m
    except Exception:
        pass


# ---------------- host-side graph preprocessing ----------------

def _preprocess(edge_index: np.ndarray):
    src = np.concatenate([edge_index[0], np.arange(N, dtype=np.int64)])
    dst = np.concatenate([edge_index[1], np.arange(N, dtype=np.int64)])
    deg = np.bincount(dst, minlength=N)

    # snake assignment over degree-sorted nodes -> 6250 nodes/core, balanced degree
    order = np.argsort(-deg, kind="stable")
    ncore = N // W  # 6250
    core_of_rank = np.tile(
        np.concatenate([np.arange(W), np.arange(W)[::-1]]), (ncore + 1) // 2 + 1
    )[:N]
    node_core = np.empty(N, dtype=np.int64)
    node_core[order] = core_of_rank

    # local index: within each core, order nodes by degree descending
    gpos = np.empty(N, dtype=np.int64)  # node -> global padded position
    perm_nodes = np.full(NFULL, -1, dtype=np.int64)  # position -> node (real only)
    Dg_per_core = np.zeros((W, G), dtype=np.int64)
    for c in range(W):
        nodes_c = order[core_of_rank == c]  # degree-descending already
        li = np.arange(len(nodes_c))
        gpos[nodes_c] = c * NLOC + li
        perm_nodes[c * NLOC + li] = nodes_c
        dc = deg[nodes_c]
        for g in range(G):
            seg = dc[g * P : (g + 1) * P]
            Dg_per_core[c, g] = int(seg.max()) if len(seg) else 0
    Dg = Dg_per_core.max(axis=0)  # uniform across cores (SPMD program)
    Dg = np.maximum(Dg, 1)
    offs = np.concatenate([[0], np.cumsum(Dg)])  # group column offsets
    SLOTS = int(offs[-1])

    # slot index per edge (rank among edges sharing the same dst);
    # self-loops sort first so slot 0 is always the self edge (direct DMA)
    gd = gpos[dst]
    sort_i = np.lexsort((src != dst, gd))
    gd_s = gd[sort_i]
    starts = np.concatenate([[0], np.flatnonzero(np.diff(gd_s)) + 1])
    counts = np.diff(np.concatenate([starts, [len(gd_s)]]))
    slot_s = np.arange(len(gd_s)) - np.repeat(starts, counts)
    slot = np.empty(len(gd_s), dtype=np.int64)
    slot[sort_i] = slot_s

    # scatter edges into per-core [P, SLOTS] index/mask arrays
    ecore = gd // NLOC
    lidx = gd % NLOC
    erow = lidx % P
    egrp = lidx // P
    ecol = offs[egrp] + slot
    assert (slot < Dg[egrp]).all()

    idx_arr = np.zeros((W, P, SLOTS), dtype=np.int32)
    mask_arr = np.full((W, P, SLOTS), -100.0, dtype=np.float32)
    idx_arr[ecore, erow, ecol] = gpos[src].astype(np.int32)
    mask_arr[ecore, erow, ecol] = 0.0

    return {
        "gpos": gpos,
        "perm_nodes": perm_nodes,
        "Dg": Dg,
        "offs": offs,
        "SLOTS": SLOTS,
        "idx_arr": idx_arr,
        "mask_arr": mask_arr,
    }


# ---------------- device program ----------------

def _build_program(Dg, offs, SLOTS):
    nc = bacc.Bacc(None, target_bir_lowering=False)

    # ---- external inputs ----
    xT = nc.dram_tensor("xT", [IN, NFULL], BF16, kind="ExternalInput")
    xTloc = nc.dram_tensor("xTloc", [IN, NLOC], BF16, kind="ExternalInput")
    idx_in = nc.dram_tensor("idx", [P, SLOTS], I32, kind="ExternalInput")
    mask4_in = nc.dram_tensor("mask4", [P, SLOTS * HEADS], BF16, kind="ExternalInput")
    wl0_in = nc.dram_tensor("wl0", [IN, HC], BF16, kind="ExternalInput")
    wr0_in = nc.dram_tensor("wr0", [IN, HC], BF16, kind="ExternalInput")
    wl1_in = nc.dram_tensor("wl1", [HC, HC], BF16, kind="ExternalInput")
    wr1_in = nc.dram_tensor("wr1", [HC, HC], BF16, kind="ExternalInput")
    wl2_in = nc.dram_tensor("wl2", [HC, NCLS], BF16, kind="ExternalInput")
    wr2_in = nc.dram_tensor("wr2", [HC, NCLS], BF16, kind="ExternalInput")
    sig0_in = nc.dram_tensor("sig0", [P, HC], BF16, kind="ExternalInput")
    sig1_in = nc.dram_tensor("sig1", [P, HC], BF16, kind="ExternalInput")
    sig2_in = nc.dram_tensor("sig2", [P, NCLS], BF16, kind="ExternalInput")
    inv0_in = nc.dram_tensor("inv0", [P, HC], F32, kind="ExternalInput")
    inv1_in = nc.dram_tensor("inv1", [P, HC], F32, kind="ExternalInput")
    inv2_in = nc.dram_tensor("inv2", [P, NCLS], F32, kind="ExternalInput")
    b0_in = nc.dram_tensor("b0p", [P, HC], F32, kind="ExternalInput")  # b*|att|
    b1_in = nc.dram_tensor("b1p", [P, HC], F32, kind="ExternalInput")
    b2_in = nc.dram_tensor("b2p", [P, NCLS], F32, kind="ExternalInput")

    out_ext = nc.dram_tensor("out", [NLOC, NCLS], F32, kind="ExternalOutput")

    import os as _os
    _dbg = _os.environ.get("KERNEL_DEBUG", "0") in ("1", "2")
    if _dbg:
        dbg_xf0 = nc.dram_tensor("dbg_xf0", [NFULL, HC], BF16, kind="ExternalOutput")
        dbg_xr0 = nc.dram_tensor("dbg_xr0", [NLOC, HC], BF16, kind="ExternalOutput")
        dbg_h1T = nc.dram_tensor("dbg_h1T", [HC, NLOC], BF16, kind="ExternalOutput")
        dbg_xf1 = nc.dram_tensor("dbg_xf1", [NFULL, HC], BF16, kind="ExternalOutput")
        dbg_h2T = nc.dram_tensor("dbg_h2T", [HC, NLOC], BF16, kind="ExternalOutput")
        dbg_xf2 = nc.dram_tensor("dbg_xf2", [NFULL, NCLS], BF16, kind="ExternalOutput")

    # ---- internal DRAM ----
    xf0 = nc.dram_tensor("xf0", [NFULL, HC], BF16)  # xl0' full (replicated compute)
    xr0 = nc.dram_tensor("xr0", [NLOC, HC], BF16)
    xl0loc = nc.dram_tensor("xl0loc", [NLOC, HC], BF16)
    h1T = nc.dram_tensor("h1T", [HC, NLOC], BF16)
    xl1loc = nc.dram_tensor("xl1loc", [NLOC, HC], BF16)
    xf1 = nc.dram_tensor("xf1", [NFULL, HC], BF16, addr_space="Shared")
    xr1 = nc.dram_tensor("xr1", [NLOC, HC], BF16)
    h2T = nc.dram_tensor("h2T", [HC, NLOC], BF16)
    xl2loc = nc.dram_tensor("xl2loc", [NLOC, NCLS], BF16)
    xf2 = nc.dram_tensor("xf2", [NFULL, NCLS], BF16, addr_space="Shared")
    xr2 = nc.dram_tensor("xr2", [NLOC, NCLS], BF16)

    rg = [list(range(W))]

    with TileContext(nc) as tc:
        with (
            tc.tile_pool(name="const", bufs=1) as cpool,
            tc.tile_pool(name="proj", bufs=6) as ppool,
            tc.tile_pool(name="edge", bufs=4) as epool,
            tc.tile_pool(name="gath", bufs=6) as gpool,
            tc.tile_pool(name="acc", bufs=2) as apool,
            tc.tile_pool(name="psum", bufs=4, space="PSUM") as psum,
            tc.tile_pool(name="psumT", bufs=2, space="PSUM") as psumT,
        ):
            # ---- resident constants ----
            ident = cpool.tile([P, P], BF16)
            make_identity(nc, ident[:])

            def load_w(handle, kdim, ndim):
                tiles = []
                for k in range(0, kdim, P):
                    t = cpool.tile([P, ndim], BF16, tag=f"w_{handle.name}_{k}")
                    nc.sync.dma_start(out=t[:], in_=handle[k : k + P, :])
                    tiles.append(t)
                return tiles

            wl0_sb = load_w(wl0_in, IN, HC)
            wr0_sb = load_w(wr0_in, IN, HC)
            wl1_sb = load_w(wl1_in, HC, HC)
            wr1_sb = load_w(wr1_in, HC, HC)
            wl2_sb = load_w(wl2_in, HC, NCLS)
            wr2_sb = load_w(wr2_in, HC, NCLS)

            def bcast_row(handle, ndim, dt, tag):
                t = cpool.tile([P, ndim], dt, tag=tag)
                nc.sync.dma_start(out=t[:], in_=handle[:, :])
                return t

            sig0_t = bcast_row(sig0_in, HC, BF16, "sig0")
            sig1_t = bcast_row(sig1_in, HC, BF16, "sig1")
            sig2_t = bcast_row(sig2_in, NCLS, BF16, "sig2")
            inv0_t = bcast_row(inv0_in, HC, F32, "inv0")
            inv1_t = bcast_row(inv1_in, HC, F32, "inv1")
            inv2_t = bcast_row(inv2_in, NCLS, F32, "inv2")
            b0_t = bcast_row(b0_in, HC, F32, "b0")
            b1_t = bcast_row(b1_in, HC, F32, "b1")
            b2_t = bcast_row(b2_in, NCLS, F32, "b2")

            # ---- helper: projection of a span of node tiles ----
            def proj_span(srcT, t0, S, w_tiles_list, nout, out_drams):
                """lhsT chunks from srcT[:, t0:t0+S] (S a multiple of P); for
                each W, matmul-accumulate per 128-node subtile, write bf16
                tiles to out_drams. Returns write instrs per (dram, subtile)."""
                nk = len(w_tiles_list[0])
                lhs = []
                for k in range(nk):
                    a = ppool.tile([P, S], BF16, tag="lhs")
                    nc.sync.dma_start(
                        out=a[:], in_=srcT[k * P : (k + 1) * P, t0 : t0 + S]
                    )
                    lhs.append(a)
                winstrs = [[] for _ in out_drams]
                for j in range(S // P):
                    for wi, (w_tiles, odram) in enumerate(zip(w_tiles_list, out_drams)):
                        ps = psum.tile([P, nout], F32, tag="proj_ps")
                        for k, (a, wk) in enumerate(zip(lhs, w_tiles)):
                            nc.tensor.matmul(
                                ps[:], a[:, j * P : (j + 1) * P], wk[:],
                                start=(k == 0), stop=(k == nk - 1),
                            )
                        ob = ppool.tile([P, nout], BF16, tag="proj_out")
                        nc.scalar.copy(ob[:], ps[:])
                        winstrs[wi].append(
                            nc.sync.dma_start(
                                out=odram[t0 + j * P : t0 + (j + 1) * P, :], in_=ob[:]
                            )
                        )
                return winstrs

            # ---- phase A: layer-0 projections ----
            SPAN = 512
            for t0 in range(0, NFULL, SPAN):
                proj_span(xT, t0, min(SPAN, NFULL - t0), [wl0_sb], HC, [xf0])
            for t0 in range(0, NLOC, SPAN):
                proj_span(
                    xTloc, t0, min(SPAN, NLOC - t0), [wr0_sb, wl0_sb], HC, [xr0, xl0loc]
                )

            # ---- edge phase (shared for layers 0/1) ----
            DMAXG = int(Dg.max())

            def edge_phase(xf, xr, xloc, sig_t, inv_t, b_t, hT_out,
                           cc_dep=None, xr_deps=None, xloc_deps=None):
                """GATv2 message passing over dst groups; writes elu'd,
                transposed h tiles to hT_out."""
                for g in range(G):
                    D = int(Dg[g])
                    off = int(offs[g])
                    xr_t = epool.tile([P, HC], BF16, tag="xr")
                    _ld = nc.sync.dma_start(out=xr_t[:], in_=xr[g * P : (g + 1) * P, :])
                    if xr_deps is not None:
                        add_dep_helper(_ld.ins, xr_deps[g].ins, True, "xr RAW")
                    idxg = epool.tile([P, DMAXG], I32, tag="idxg")
                    nc.sync.dma_start(out=idxg[:, :D], in_=idx_in[:, off : off + D])
                    mkg = epool.tile([P, DMAXG * HEADS], BF16, tag="mkg")
                    nc.sync.dma_start(
                        out=mkg[:, : D * HEADS],
                        in_=mask4_in[:, off * HEADS : (off + D) * HEADS],
                    )
                    accs = apool.tile([P, B * HC], BF16, tag="accs")
                    nc.gpsimd.memset(accs[:], 0.0)
                    den = apool.tile([P, HEADS * B], F32, tag="den")
                    nc.gpsimd.memset(den[:], 0.0)

                    for d0 in range(0, D, B):
                        b = min(B, D - d0)
                        xg = gpool.tile([P, B * HC], BF16, tag="xg")
                        for bb in range(b):
                            if d0 == 0 and bb == 0:
                                # slot 0 is the self-loop: contiguous local rows
                                _gi = nc.sync.dma_start(
                                    out=xg[:, 0:HC],
                                    in_=xloc[g * P : (g + 1) * P, :],
                                )
                                if xloc_deps is not None:
                                    add_dep_helper(
                                        _gi.ins, xloc_deps[g].ins, True, "selfloop RAW"
                                    )
                                continue
                            _gi = nc.gpsimd.indirect_dma_start(
                                out=xg[:, bb * HC : (bb + 1) * HC],
                                out_offset=None,
                                in_=xf[:, :],
                                in_offset=bass.IndirectOffsetOnAxis(
                                    ap=idxg[:, d0 + bb : d0 + bb + 1], axis=0
                                ),
                            )
                            if cc_dep is not None:
                                add_dep_helper(_gi.ins, cc_dep.ins, True, "gather RAW cc")
                        # q = xg + xr (broadcast over slots)
                        q = epool.tile([P, B * HC], BF16, tag="q")
                        nc.vector.tensor_tensor(
                            out=q[:, : b * HC].rearrange("p (b c) -> p b c", b=b),
                            in0=xg[:, : b * HC].rearrange("p (b c) -> p b c", b=b),
                            in1=xr_t[:, None, :].to_broadcast([P, b, HC]),
                            op=mybir.AluOpType.add,
                        )
                        # v = prelu(q, 0.2) on ACT
                        v = epool.tile([P, B * HC], BF16, tag="v")
                        nc.scalar.activation(
                            v[:, : b * HC], q[:, : b * HC],
                            mybir.ActivationFunctionType.Prelu, alpha=0.2,
                        )
                        # w = v * sign(att), in place
                        nc.vector.tensor_tensor(
                            out=v[:, : b * HC].rearrange("p (b c) -> p b c", b=b),
                            in0=v[:, : b * HC].rearrange("p (b c) -> p b c", b=b),
                            in1=sig_t[:, None, :].to_broadcast([P, b, HC]),
                            op=mybir.AluOpType.mult,
                        )
                        # alpha[p, b, h] = sum_c w
                        al = epool.tile([P, B * HEADS], F32, tag="al")
                        nc.vector.tensor_reduce(
                            out=al[:, : b * HEADS].rearrange("p (b h) -> p b h", h=HEADS),
                            in_=v[:, : b * HC].rearrange(
                                "p (b h c) -> p b h c", h=HEADS, c=HID
                            ),
                            axis=mybir.AxisListType.X,
                            op=mybir.AluOpType.add,
                        )
                        # alpha += mask (from the group-resident mask tile)
                        nc.vector.tensor_tensor(
                            out=al[:, : b * HEADS], in0=al[:, : b * HEADS],
                            in1=mkg[:, d0 * HEADS : (d0 + b) * HEADS],
                            op=mybir.AluOpType.add,
                        )
                        # p = exp(alpha), replicated across channels (ACT
                        # broadcast-in writes each exp 64x -> matches q layout)
                        pr = epool.tile([P, B * HC], BF16, tag="pr")
                        nc.scalar.activation(
                            pr[:, : b * HC].rearrange("p (bh c) -> p bh c", c=HID),
                            al[:, : b * HEADS][:, :, None].to_broadcast(
                                [P, b * HEADS, HID]
                            ),
                            mybir.ActivationFunctionType.Exp,
                        )
                        # den[h, b] += p[b, h] from the replicated tile (stride HID)
                        nc.vector.tensor_tensor(
                            out=den[:].rearrange("p (h b) -> p h b", h=HEADS)[:, :, :b],
                            in0=den[:].rearrange("p (h b) -> p h b", h=HEADS)[:, :, :b],
                            in1=pr[:, : b * HC]
                            .rearrange("p (b h c) -> p b h c", h=HEADS, c=HID)[
                                :, :, :, 0:1
                            ]
                            .rearrange("p b h x -> p h (b x)"),
                            op=mybir.AluOpType.add,
                        )
                        # t = q * p  (both bf16 unit-stride -> 2x mode)
                        tt = epool.tile([P, B * HC], BF16, tag="tt")
                        nc.vector.tensor_tensor(
                            out=tt[:, : b * HC], in0=xg[:, : b * HC],
                            in1=pr[:, : b * HC], op=mybir.AluOpType.mult,
                        )
                        # slot-wise accumulate (bf16 2x mode)
                        nc.vector.tensor_tensor(
                            out=accs[:, : b * HC], in0=accs[:, : b * HC],
                            in1=tt[:, : b * HC], op=mybir.AluOpType.add,
                        )

                    # ---- group end ----
                    acc = apool.tile([P, HC], F32, tag="acc")
                    nc.vector.tensor_reduce(
                        out=acc[:].rearrange("p (c x) -> p c x", x=1),
                        in_=accs[:].rearrange("p (b c) -> p c b", b=B),
                        axis=mybir.AxisListType.X,
                        op=mybir.AluOpType.add,
                    )
                    dsum = apool.tile([P, HEADS], F32, tag="dsum")
                    nc.vector.tensor_reduce(
                        out=dsum[:].rearrange("p (h x) -> p h x", x=1),
                        in_=den[:].rearrange("p (h b) -> p h b", h=HEADS),
                        axis=mybir.AxisListType.X,
                        op=mybir.AluOpType.add,
                    )
                    nc.vector.tensor_scalar_add(dsum[:], dsum[:], 1e-16)
                    rec = apool.tile([P, HEADS], F32, tag="rec")
                    nc.vector.reciprocal(rec[:], dsum[:])
                    on = apool.tile([P, HC], F32, tag="on")
                    nc.vector.tensor_tensor(
                        out=on[:].rearrange("p (h c) -> p h c", h=HEADS),
                        in0=acc[:].rearrange("p (h c) -> p h c", h=HEADS),
                        in1=rec[:, :, None].to_broadcast([P, HEADS, HID]),
                        op=mybir.AluOpType.mult,
                    )
                    # + b*|att|, then / |att|
                    nc.vector.tensor_tensor(
                        out=on[:], in0=on[:], in1=b_t[:], op=mybir.AluOpType.add
                    )
                    nc.vector.tensor_tensor(
                        out=on[:], in0=on[:], in1=inv_t[:], op=mybir.AluOpType.mult
                    )
                    # elu -> bf16 h tile
                    mn = apool.tile([P, HC], F32, tag="mn")
                    nc.vector.tensor_scalar_min(mn[:], on[:], 0.0)
                    em = apool.tile([P, HC], F32, tag="em")
                    nc.scalar.activation(em[:], mn[:], mybir.ActivationFunctionType.Exp)
                    rl = apool.tile([P, HC], F32, tag="rl")
                    nc.vector.tensor_scalar_max(rl[:], on[:], 0.0)
                    hb = apool.tile([P, HC], BF16, tag="hb")
                    nc.vector.scalar_tensor_tensor(
                        out=hb[:], in0=em[:], scalar=-1.0, in1=rl[:],
                        op0=mybir.AluOpType.add, op1=mybir.AluOpType.add,
                    )
                    # transpose h tile -> hT_out[:, g*P:(g+1)*P]
                    for k in range(HC // P):
                        tp = psumT.tile([P, P], BF16, tag="tp")
                        nc.tensor.transpose(
                            out=tp[:], in_=hb[:, k * P : (k + 1) * P], identity=ident[:]
                        )
                        tb = ppool.tile([P, P], BF16, tag="tb")
                        nc.scalar.copy(tb[:], tp[:])
                        nc.sync.dma_start(
                            out=hT_out[k * P : (k + 1) * P, g * P : (g + 1) * P],
                            in_=tb[:],
                        )

            # ---- barrier: xf0/xr0 writes -> edge-phase gathers ----
            tc.strict_bb_all_engine_barrier()

            # ---- layer 0 ----
            edge_phase(xf0, xr0, xl0loc, sig0_t, inv0_t, b0_t, h1T)

            # ---- barrier: h1T writes -> layer-1 projection reads ----
            tc.strict_bb_all_engine_barrier()

            # ---- layer 1 projections + AllGather ----
            xr1_w = []
            xl1_w = []
            for t0 in range(0, NLOC, SPAN):
                ws = proj_span(
                    h1T, t0, min(SPAN, NLOC - t0), [wl1_sb, wr1_sb], HC, [xl1loc, xr1]
                )
                xl1_w.extend(ws[0])
                xr1_w.extend(ws[1])
            cc1 = nc.gpsimd.collective_compute(
                "AllGather", mybir.AluOpType.bypass, replica_groups=rg,
                ins=[xl1loc[:, :]], outs=[xf1[:, :]],
            )

            # ---- layer 1 ----
            edge_phase(xf1, xr1, xl1loc, sig1_t, inv1_t, b1_t, h2T,
                       cc_dep=cc1, xr_deps=xr1_w, xloc_deps=xl1_w)

            # ---- barrier: h2T writes -> layer-2 projection reads ----
            tc.strict_bb_all_engine_barrier()

            # ---- layer 2 projections + AllGather ----
            xr2_w = []
            xl2_w = []
            for t0 in range(0, NLOC, SPAN):
                ws = proj_span(
                    h2T, t0, min(SPAN, NLOC - t0), [wl2_sb, wr2_sb], NCLS, [xl2loc, xr2]
                )
                xl2_w.extend(ws[0])
                xr2_w.extend(ws[1])
            cc2 = nc.gpsimd.collective_compute(
                "AllGather", mybir.AluOpType.bypass, replica_groups=rg,
                ins=[xl2loc[:, :]], outs=[xf2[:, :]],
            )

            # ---- layer 2 edge phase (H=1, C=2) ----
            for g in range(G):
                D = int(Dg[g])
                off = int(offs[g])
                xr_t = epool.tile([P, NCLS], BF16, tag="xr2")
                _ld2 = nc.sync.dma_start(out=xr_t[:], in_=xr2[g * P : (g + 1) * P, :])
                add_dep_helper(_ld2.ins, xr2_w[g].ins, True, "xr2 RAW")
                idxg2 = epool.tile([P, DMAXG], I32, tag="idxg2")
                nc.sync.dma_start(out=idxg2[:, :D], in_=idx_in[:, off : off + D])
                mkg2 = epool.tile([P, DMAXG], BF16, tag="mkg2")
                nc.sync.dma_start(
                    out=mkg2[:, :D],
                    in_=mask4_in[:]
                    .rearrange("p (s h) -> p s h", h=HEADS)[:, off : off + D, 0:1]
                    .rearrange("p s x -> p (s x)"),
                )
                acc2 = apool.tile([P, NCLS * B2], F32, tag="acc2")
                nc.gpsimd.memset(acc2[:], 0.0)
                den2 = apool.tile([P, B2], F32, tag="den2")
                nc.gpsimd.memset(den2[:], 0.0)

                for d0 in range(0, D, B2):
                    b = min(B2, D - d0)
                    xg = gpool.tile([P, B2 * NCLS], BF16, tag="xg2")
                    for bb in range(b):
                        if d0 == 0 and bb == 0:
                            _gi2 = nc.sync.dma_start(
                                out=xg[:, 0:NCLS],
                                in_=xl2loc[g * P : (g + 1) * P, :],
                            )
                            add_dep_helper(_gi2.ins, xl2_w[g].ins, True, "selfloop2 RAW")
                            continue
                        _gi2 = nc.gpsimd.indirect_dma_start(
                            out=xg[:, bb * NCLS : (bb + 1) * NCLS],
                            out_offset=None,
                            in_=xf2[:, :],
                            in_offset=bass.IndirectOffsetOnAxis(
                                ap=idxg2[:, d0 + bb : d0 + bb + 1], axis=0
                            ),
                        )
                        add_dep_helper(_gi2.ins, cc2.ins, True, "gather RAW cc2")
                    q2 = epool.tile([P, B2 * NCLS], BF16, tag="q2")
                    nc.vector.tensor_tensor(
                        out=q2[:, : b * NCLS].rearrange("p (b c) -> p b c", b=b),
                        in0=xg[:, : b * NCLS].rearrange("p (b c) -> p b c", b=b),
                        in1=xr_t[:, None, :].to_broadcast([P, b, NCLS]),
                        op=mybir.AluOpType.add,
                    )
                    v = epool.tile([P, B2 * NCLS], BF16, tag="v2")
                    nc.scalar.activation(
                        v[:, : b * NCLS], q2[:, : b * NCLS],
                        mybir.ActivationFunctionType.Prelu, alpha=0.2,
                    )
                    nc.vector.tensor_tensor(
                        out=v[:, : b * NCLS].rearrange("p (b c) -> p b c", b=b),
                        in0=v[:, : b * NCLS].rearrange("p (b c) -> p b c", b=b),
                        in1=sig2_t[:, None, :].to_broadcast([P, b, NCLS]),
                        op=mybir.AluOpType.mult,
                    )
                    al = epool.tile([P, B2], F32, tag="al2")
                    nc.vector.tensor_reduce(
                        out=al[:, :b].rearrange("p (b x) -> p b x", x=1),
                        in_=v[:, : b * NCLS].rearrange("p (b c) -> p b c", b=b),
                        axis=mybir.AxisListType.X,
                        op=mybir.AluOpType.add,
                    )
                    nc.vector.tensor_tensor(
                        out=al[:, :b], in0=al[:, :b],
                        in1=mkg2[:, d0 : d0 + b],
                        op=mybir.AluOpType.add,
                    )
                    pe = epool.tile([P, B2], F32, tag="pe2")
                    nc.scalar.activation(
                        pe[:, :b], al[:, :b], mybir.ActivationFunctionType.Exp
                    )
                    nc.vector.tensor_tensor(
                        out=den2[:, :b], in0=den2[:, :b], in1=pe[:, :b],
                        op=mybir.AluOpType.add,
                    )
                    # acc2[c, b] += xg[b, c] * p[b]
                    tt = epool.tile([P, B2 * NCLS], F32, tag="tt2")
                    nc.vector.tensor_tensor(
                        out=tt[:, : b * NCLS].rearrange("p (b c) -> p b c", b=b),
                        in0=xg[:, : b * NCLS].rearrange("p (b c) -> p b c", b=b),
                        in1=pe[:, :b, None].to_broadcast([P, b, NCLS]),
                        op=mybir.AluOpType.mult,
                    )
                    nc.vector.tensor_tensor(
                        out=acc2[:].rearrange("p (c b) -> p c b", c=NCLS)[:, :, :b],
                        in0=acc2[:].rearrange("p (c b) -> p c b", c=NCLS)[:, :, :b],
                        in1=tt[:, : b * NCLS].rearrange("p (b c) -> p c b", b=b),
                        op=mybir.AluOpType.add,
                    )

                dsum = apool.tile([P, 1], F32, tag="dsum2")
                nc.vector.tensor_reduce(
                    out=dsum[:], in_=den2[:], axis=mybir.AxisListType.X,
                    op=mybir.AluOpType.add,
                )
                nc.vector.tensor_scalar_add(dsum[:], dsum[:], 1e-16)
                rec = apool.tile([P, 1], F32, tag="rec2")
                nc.vector.reciprocal(rec[:], dsum[:])
                o2 = apool.tile([P, NCLS], F32, tag="o2")
                nc.vector.tensor_reduce(
                    out=o2[:].rearrange("p (c x) -> p c x", x=1),
                    in_=acc2[:].rearrange("p (c b) -> p c b", c=NCLS),
                    axis=mybir.AxisListType.X,
                    op=mybir.AluOpType.add,
                )
                nc.vector.tensor_scalar(
                    out=o2[:], in0=o2[:], scalar1=rec[:, :1], scalar2=None,
                    op0=mybir.AluOpType.mult,
                )
                nc.vector.tensor_tensor(
                    out=o2[:], in0=o2[:], in1=b2_t[:], op=mybir.AluOpType.add
                )
                nc.vector.tensor_tensor(
                    out=o2[:], in0=o2[:], in1=inv2_t[:], op=mybir.AluOpType.mult
                )
                nc.sync.dma_start(out=out_ext[g * P : (g + 1) * P, :], in_=o2[:])

            if _dbg:
                nc.sync.dma_start(out=dbg_xf0[:, :], in_=xf0[:, :])
                nc.sync.dma_start(out=dbg_xr0[:, :], in_=xr0[:, :])
                nc.sync.dma_start(out=dbg_h1T[:, :], in_=h1T[:, :])
                _dx1 = nc.sync.dma_start(out=dbg_xf1[:, :], in_=xf1[:, :])
                add_dep_helper(_dx1.ins, cc1.ins, True, "dbg xf1")
                nc.sync.dma_start(out=dbg_h2T[:, :], in_=h2T[:, :])
                _dx2 = nc.sync.dma_start(out=dbg_xf2[:, :], in_=xf2[:, :])
                add_dep_helper(_dx2.ins, cc2.ins, True, "dbg xf2")

    nc.compile()
    return nc


# ---------------- top-level entry ----------------

def kernel(**inputs) -> np.ndarray:
    _install_profile_hook()

    x = np.asarray(inputs["x"], dtype=np.float32)
    edge_index = np.asarray(inputs["edge_index"])
    pre = _preprocess(np.asarray(edge_index, dtype=np.int64))

    gpos, perm_nodes = pre["gpos"], pre["perm_nodes"]
    Dg, offs, SLOTS = pre["Dg"], pre["offs"], pre["SLOTS"]
    idx_arr, mask_arr = pre["idx_arr"], pre["mask_arr"]

    # host-side weight folding
    def fold(att):
        a = np.asarray(att, dtype=np.float32).reshape(-1)
        mag = np.abs(a)
        sig = np.sign(a)
        bad = mag < 1e-12
        mag[bad] = 1.0
        sig[bad] = 0.0
        return mag, sig

    mag0, sg0 = fold(inputs["att0"])
    mag1, sg1 = fold(inputs["att1"])
    mag2, sg2 = fold(inputs["att2"])

    wl0 = (np.asarray(inputs["Wl0"], np.float32) * mag0[None, :]).astype(NPBF16)
    wr0 = (np.asarray(inputs["Wr0"], np.float32) * mag0[None, :]).astype(NPBF16)
    wl1 = (np.asarray(inputs["Wl1"], np.float32) * mag1[None, :]).astype(NPBF16)
    wr1 = (np.asarray(inputs["Wr1"], np.float32) * mag1[None, :]).astype(NPBF16)
    wl2 = (np.asarray(inputs["Wl2"], np.float32) * mag2[None, :]).astype(NPBF16)
    wr2 = (np.asarray(inputs["Wr2"], np.float32) * mag2[None, :]).astype(NPBF16)

    b0p = (np.asarray(inputs["b0"], np.float32) * mag0)[None, :].astype(np.float32)
    b1p = (np.asarray(inputs["b1"], np.float32) * mag1)[None, :].astype(np.float32)
    b2p = (np.asarray(inputs["b2"], np.float32) * mag2)[None, :].astype(np.float32)

    # permuted, padded, transposed input
    x_perm = np.zeros((NFULL, IN), dtype=np.float32)
    real = perm_nodes >= 0
    x_perm[real] = x[perm_nodes[real]]
    xT_np = np.ascontiguousarray(x_perm.T).astype(NPBF16)

    common = {
        "xT": xT_np,
        "wl0": wl0, "wr0": wr0, "wl1": wl1, "wr1": wr1, "wl2": wl2, "wr2": wr2,
        "sig0": np.broadcast_to(sg0[None, :], (P, HC)).astype(NPBF16),
        "sig1": np.broadcast_to(sg1[None, :], (P, HC)).astype(NPBF16),
        "sig2": np.broadcast_to(sg2[None, :], (P, NCLS)).astype(NPBF16),
        "inv0": np.ascontiguousarray(np.broadcast_to((1.0 / mag0)[None, :], (P, HC))).astype(np.float32),
        "inv1": np.ascontiguousarray(np.broadcast_to((1.0 / mag1)[None, :], (P, HC))).astype(np.float32),
        "inv2": np.ascontiguousarray(np.broadcast_to((1.0 / mag2)[None, :], (P, NCLS))).astype(np.float32),
        "b0p": np.ascontiguousarray(np.broadcast_to(b0p, (P, HC))).astype(np.float32),
        "b1p": np.ascontiguousarray(np.broadcast_to(b1p, (P, HC))).astype(np.float32),
        "b2p": np.ascontiguousarray(np.broadcast_to(b2p, (P, NCLS))).astype(np.float32),
    }
    in_maps = []
    for c in range(W):
        m = dict(common)
        m["xTloc"] = np.ascontiguousarray(xT_np[:, c * NLOC : (c + 1) * NLOC])
        m["idx"] = idx_arr[c]
        m["mask4"] = np.repeat(mask_arr[c], HEADS, axis=1).astype(NPBF16)
        in_maps.append(m)

    nc = _build_program(Dg, offs, SLOTS)
    import os
    trace = os.environ.get("KERNEL_TRACE", "0") == "1"
    res = run_bass_kernel_spmd(nc, in_maps, list(range(W)), trace=trace)
    kernel.last_exec_time_ns = res.exec_time_ns
    kernel.last_results = res.results
    kernel.last_pre = pre
    kernel.last_in_maps = in_maps

    out_full = np.zeros((N, NCLS), dtype=np.float32)
    for c in range(W):
        o = res.results[c]["out"]  # [NLOC, 2]
        sel = perm_nodes[c * NLOC : (c + 1) * NLOC]
        r = sel >= 0
        out_full[sel[r]] = o[r]
    return out_full


if __name__ == "__main__":
    rng = np.random.default_rng(0)
    x = rng.standard_normal((N, IN)).astype(np.float32)
    ei = rng.integers(0, N, (2, E)).astype(np.int64)
    print("smoke build only")
